# revision 30
# baseline (speedup 1.0000x reference)
"""Trainium2 Bass kernel for nn_Conv2d_lsq_int (LSQ int8-style quantized 3x3 conv).

Full-input contract: kernel(**inputs) takes the complete tensors
(x[16,320,64,64], weight[320,320,3,3], bias[320], scalar step sizes) and
returns the full [16,320,64,64] float32 output.

Distribution: data-parallel over the batch dim — 2 images per NeuronCore on
8 cores; weight/bias replicated. The host only shards the batch, quantizes +
Winograd-transforms the weight (pure weight preprocessing), computes the
320-element bias requant, and concatenates the per-core outputs.

Algorithm: 1D Winograd F(2,3) along the width dimension, fp16 operands.
 - x_int/w_int are integers in [-127,127]. Weight tap-rows g are transformed
   with 2*G = [[2,0,0],[1,1,1],[1,-1,1],[0,0,2]] -> U[kh,j] (ints, |U|<=381).
   Input rows are transformed with B^T -> R[j, tw] = [d0-d2, d1+d2, d2-d1,
   d1-d3] on even/odd column pairs (ints, |R|<=254). Both are exact in fp16
   (integers < 2048). Per output row-pair column tw: y[2tw+b] =
   (A^T M)[b] / 2 where M[j] = sum_{ci,kh} U[ci,kh,j] * R[ci, j, r+kh, tw].
 - MAC count per output: 4 j * 320 ci * 3 kh / 2 outputs = 1920 vs direct
   2880 -> 1.5x fewer tensor cycles. Products <= 97k and accumulated sums
   < 2^24, so fp32 PSUM accumulation is exact and the result matches the
   reference bit-for-bit (validated in numpy).
 - R layout [p, j, r, tw]: the GEMM moving operand (fixed j, 16 rows x 32
   tw) is one fully contiguous 1 KiB run per partition.
 - Contraction per (j, psum tile) = (ci 320 x kh 3) = 960 = 7.5*128: two full
   128-ci chunks x 3 kh, plus a 64-ci remainder packed as kh-pairs via a
   row-shifted partition-duplicated R copy (made by an SBUF DMA), plus K=64
   singles run pairwise concurrent via PE row tiling.
 - round() is fp32 add of 1.5*2**23 (round-to-nearest-even) fused into ACT
   activation ops; the epilogue clip+bias+clip collapses to a 2-op
   per-partition clamp (clip(clip(r)+b) == clamp(r+b, lo(b), hi(b))).
 - Engine budget: engines read at most one PSUM operand per instruction and
   gpsimd none, so ACT stages M1/M2 to SBUF, DVE does the PSUM-touching
   ops + R transform, gpsimd the SBUF-only combines.
"""

import contextlib
import ctypes
import sys
import types

import numpy as np

import concourse.bass as bass  # noqa: F401
import concourse.tile as tile
from concourse import bacc, mybir
from concourse.bass_utils import run_bass_kernel_spmd

F32 = mybir.dt.float32
F16 = mybir.dt.float16
OP = mybir.AluOpType
ACTF = mybir.ActivationFunctionType

MAGIC = 12582912.0  # 1.5 * 2**23 : fp32 round-to-nearest-even trick
QMAX = 127.0

B, CIN, COUT, H, W, K = 16, 320, 320, 64, 64, 3
N_CORES = 8
IMGS_PER_CORE = B // N_CORES
HW = H * W
J = 4            # winograd positions along width
TW = 32          # output column tiles (2 cols each)
PR = H + 2       # padded row count for R
RB = 16          # output rows per psum tile (N = RB*TW = 512)
NBLK = H // RB   # 4 row blocks per image
SLAB = 16        # x load/quant pipelined in 16-row slabs
CHUNKS = [(0, 128), (128, 128), (256, 64)]  # (start, size) along cin / cout
NSLOT = K * J    # 12 weight slots (kh, j)


def _install_axon_ntff_hook():
    """Slim antenv.axon_hooks so trace=True works (and never crashes) here."""
    if "antenv.axon_hooks" in sys.modules:
        return
    hook = None
    try:
        lib = ctypes.CDLL("/opt/axon/libaxon_pjrt.so")
        if hasattr(lib, "axon_start_nrt_profile"):
            lib.axon_start_nrt_profile.argtypes = [
                ctypes.POINTER(ctypes.c_int64),
                ctypes.c_size_t,
            ]
            lib.axon_start_nrt_profile.restype = ctypes.c_int64
            lib.axon_stop_nrt_profile.argtypes = [ctypes.c_char_p]
            lib.axon_stop_nrt_profile.restype = ctypes.c_int64

            @contextlib.contextmanager
            def hook(output_dir, device_ids):  # noqa: F811
                import jax

                jax.devices()
                if device_ids:
                    ids = (ctypes.c_int64 * len(device_ids))(*device_ids)
                    rc = lib.axon_start_nrt_profile(ids, len(device_ids))
                else:
                    rc = lib.axon_start_nrt_profile(None, 0)
                if rc != 0:
                    raise RuntimeError(f"axon_start_nrt_profile rc={rc}")
                try:
                    yield
                finally:
                    n = lib.axon_stop_nrt_profile(str(output_dir).encode())
                    print(f"profile: {n} ntff file(s) -> {output_dir}",
                          file=sys.stderr)
    except OSError:
        pass

    mod = types.ModuleType("antenv.axon_hooks")
    mod.get_axon_ntff_profile_hook = lambda: hook
    mod.set_axon_ntff_profile_hook = lambda h: None
    sys.modules["antenv.axon_hooks"] = mod

    # keep profiling artifacts local (zero-egress container)
    import concourse.bass_utils as bu

    bu.upload_artifacts = lambda tmpdir: "local://" + str(tmpdir)


def bias_int8(b, sb, ss, sx, sw):
    """Host fp32 replica of the reference's bias requant (DVE lacks divide).

    Every op is a single IEEE-754 fp32 operation in the reference's exact
    order, so this is bit-identical to the jax fp32 computation.
    """
    f32 = np.float32
    b = np.asarray(b, np.float32)
    b_deq = np.clip(np.round(b / f32(sb)), -QMAX, QMAX).astype(np.float32) * f32(sb)
    x_scale = f32(1.0) / f32(sx)
    w_scale = f32(1.0) / f32(sw)
    t = ((b_deq * f32(ss)) * x_scale) * w_scale
    return np.clip(np.round(t), -QMAX, QMAX).astype(np.float32)


def bias_params(b_i8):
    """Fused epilogue clamp params: clip(clip(r,+-127)+b, +-127) ==
    clamp(r+b, -127+max(0,b), 127+min(0,b)) since |b|<=127. Shipped as
    [128, 9]: cols p*3+c for param p in (MAGIC-b, hi, lo) and cout chunk
    c in (0:128, 128:256, 256:320 tiled on both partition halves)."""
    cols = np.zeros((128, 9), np.float32)
    vs = [b_i8[0:128], b_i8[128:256], np.tile(b_i8[256:320], 2)]
    for c, v in enumerate(vs):
        cols[:, 0 * 3 + c] = np.float32(MAGIC) - v
        cols[:, 1 * 3 + c] = QMAX + np.minimum(0.0, v)
        cols[:, 2 * 3 + c] = -QMAX + np.maximum(0.0, v)
    return cols


def prep_u(w, sw):
    """Host weight preprocessing: quantize + 1D Winograd transform (2G over
    kw), laid out [ci, (kh, j, cout)] in fp16 (all values are ints <= 381,
    exactly representable)."""
    f32 = np.float32
    w = np.asarray(w, np.float32)
    w_int = np.clip(np.round(w / f32(sw)), -QMAX, QMAX).astype(np.float32)
    g0, g1, g2 = w_int[..., 0], w_int[..., 1], w_int[..., 2]
    # [o, ci, kh, j]
    u = np.stack([2 * g0, g0 + g1 + g2, g0 - g1 + g2, 2 * g2], axis=-1)
    # -> [ci, kh, j, o] -> [CIN, 12*COUT]
    u = np.ascontiguousarray(u.transpose(1, 2, 3, 0)).reshape(CIN, NSLOT * COUT)
    return u.astype(np.float16)


def _build(sx: float, sw: float, sb: float, ss: float):
    """Build the per-core Bass program. Scalars are baked as immediates."""
    nc = bacc.Bacc("TRN2", target_bir_lowering=False, debug=False)

    x_d = nc.dram_tensor("x", [IMGS_PER_CORE, CIN, HW], F32, kind="ExternalInput")
    u_d = nc.dram_tensor("u", [CIN, NSLOT * COUT], F16, kind="ExternalInput")
    b_d = nc.dram_tensor("b", [128, 9], F32, kind="ExternalInput")
    y_d = nc.dram_tensor("y", [IMGS_PER_CORE, COUT, HW], F32, kind="ExternalOutput")

    r_x = float(np.float32(1.0) / np.float32(sx))  # x_scale
    # epilogue scale: shift_scale applied to y = (A^T M)/2
    ss_h = float(np.float32(ss)) / 2.0

    n_slabs = H // SLAB

    def slot(kh, j):
        return (kh * J + j) * COUT

    with tile.TileContext(nc) as tc:
        with (
            tc.tile_pool(name="persist", bufs=1) as persist,
            tc.tile_pool(name="xstage", bufs=4) as xstage,
            tc.tile_pool(name="epi", bufs=6) as epi,
            tc.tile_pool(name="otile", bufs=2) as opool,
            tc.tile_pool(name="psum", bufs=8, space="PSUM") as psum,
        ):
            # ---------------- weights: fp16 winograd U, DMA only ----------
            uq = {}
            half = (NSLOT // 2) * COUT
            for c in range(len(CHUNKS)):
                uq[c] = persist.tile(
                    [128, NSLOT * COUT], F16, tag=f"uq{c}", name=f"uq{c}"
                )

            def emit_u_chunk(c):
                ci0, pc = CHUNKS[c]
                if pc == 128:
                    for lo, hi in ((0, half), (half, NSLOT * COUT)):
                        nc.sync.dma_start(
                            uq[c][:, lo:hi], u_d[ci0 : ci0 + pc, lo:hi]
                        )
                else:
                    # parts 0:64 hold U[ci, kh, j]; parts 64:128 hold
                    # U[ci, kh+1, j] (slot-shifted) for kh-pair packing.
                    nc.sync.dma_start(
                        uq[c][:pc, :], u_d[ci0 : ci0 + pc, :]
                    )
                    nc.sync.dma_start(
                        uq[c][pc : 2 * pc, 0 : 2 * J * COUT],
                        u_d[ci0 : ci0 + pc, J * COUT : K * J * COUT],
                    )

            # ---------------- R: winograd-transformed input rows ----------
            # rq[(i,c)] viewed [p, j(4), r(66), tw(32)] fp16.
            # c=2: parts 0:64 = R rows as-is; parts 64:128 = R shifted one
            # row up (row r holds R[r+1]) for kh-pair / single packing.
            rq = {}
            for i in range(IMGS_PER_CORE):
                for c in range(len(CHUNKS)):
                    t = persist.tile(
                        [128, J * PR * TW], F16, tag=f"rq{i}_{c}",
                        name=f"rq{i}_{c}"
                    )
                    rq[(i, c)] = t.rearrange("p (j r t) -> p j r t", j=J, r=PR)

            def emit_r_memsets(i):
                for c in range(len(CHUNKS)):
                    r3 = rq[(i, c)]
                    if CHUNKS[c][1] == 128:
                        nc.gpsimd.memset(r3[:, :, 0:1, :], 0.0)
                        nc.gpsimd.memset(r3[:, :, PR - 1 :, :], 0.0)
                    else:
                        nc.gpsimd.memset(r3[0:64, :, 0:1, :], 0.0)
                        nc.gpsimd.memset(r3[0:64, :, PR - 1 :, :], 0.0)
                        nc.gpsimd.memset(r3[64:128, :, PR - 2 : PR - 1, :], 0.0)

            def emit_x_slab(i, s, only_c=None):
                """DMA a 16-row slab, quantize (round+clip in magic space),
                then write the width-transformed R rows directly. The
                remainder chunk's row-shifted duplicate (parts 64:128) is a
                partition-offset SBUF DMA copy of the computed R rows."""
                r0 = s * SLAB
                for c, (ci0, pc) in enumerate(CHUNKS):
                    if only_c is not None and c != only_c:
                        continue
                    st = xstage.tile([128, SLAB * W], F32, tag="xst",
                                     name="xst")
                    nc.sync.dma_start(
                        st[:pc, :],
                        x_d[i, ci0 : ci0 + pc, r0 * W : (r0 + SLAB) * W],
                    )
                    nc.scalar.activation(
                        st[:pc, :], st[:pc, :], ACTF.Copy, bias=MAGIC, scale=r_x
                    )
                    # clip in magic-offset space, single dual-op on DVE
                    nc.vector.tensor_scalar(
                        st[:pc, :], st[:pc, :], MAGIC + QMAX, MAGIC - QMAX,
                        OP.min, OP.max,
                    )
                    # even/odd column view: [p, r, tw(32), 2]
                    xv = st.rearrange("p (r c2 two) -> p r c2 two", two=2, c2=TW)
                    ev = lambda a, b: xv[0:pc, :, a:b, 0:1]
                    od = lambda a, b: xv[0:pc, :, a:b, 1:2]
                    d = rq[(i, c)]
                    rows = slice(1 + r0, 1 + r0 + SLAB)
                    # R on DVE (gpsimd's elementwise path is ~2.6x slower
                    # and fp16 slower still); magic offsets cancel in
                    # subtracts, j1 subtracts 2*MAGIC
                    ve, gp = nc.vector, nc.gpsimd
                    # j=0: R0[0] = -x[1]; R0[tw>=1] = x[2tw-1]-x[2tw+1]
                    ve.tensor_scalar(
                        d[0:pc, 0:1, rows, 0:1], od(0, 1),
                        -1.0, MAGIC, OP.mult, OP.add,
                    )
                    gp.tensor_tensor(
                        d[0:pc, 0:1, rows, 1:TW],
                        od(0, TW - 1), od(1, TW),
                        OP.subtract,
                    )
                    # j=1: x[2tw] + x[2tw+1]  (magic: (a-2M)+b)
                    ve.scalar_tensor_tensor(
                        d[0:pc, 1:2, rows, :],
                        ev(0, TW), 2.0 * MAGIC, od(0, TW),
                        OP.subtract, OP.add,
                    )
                    # j=2: x[2tw+1] - x[2tw]
                    ve.tensor_tensor(
                        d[0:pc, 2:3, rows, :],
                        od(0, TW), ev(0, TW),
                        OP.subtract,
                    )
                    # j=3: R3[tw<31] = x[2tw]-x[2tw+2]; R3[31] = x[62]
                    gp.tensor_tensor(
                        d[0:pc, 3:4, rows, 0 : TW - 1],
                        ev(0, TW - 1), ev(1, TW),
                        OP.subtract,
                    )
                    ve.tensor_scalar(
                        d[0:pc, 3:4, rows, TW - 1 : TW],
                        ev(TW - 1, TW), MAGIC, None, OP.subtract,
                    )
                    if pc < 128:
                        # row-shifted duplicate: R2b[r] = R[r+1]
                        nc.sync.dma_start(
                            d[64:128, :, r0 : r0 + SLAB, :],
                            d[0:64, :, 1 + r0 : 1 + r0 + SLAB, :],
                        )

            # emission order: first-consumption order — slab-0 chunk-0 DMA
            # goes out ahead of the 2.5 MB weight DMA so the quant pipeline
            # starts immediately. Image 1's slabs are emitted later,
            # interleaved between image-0 GEMM sections, so their DVE
            # quant/R work doesn't sit ahead of image-0 epilogue ops in the
            # in-order engine queues (that stalls PSUM drain).
            emit_x_slab(0, 0, only_c=0)
            emit_u_chunk(0)
            emit_x_slab(0, 0, only_c=1)
            emit_x_slab(0, 0, only_c=2)
            emit_u_chunk(1)
            emit_r_memsets(0)
            emit_x_slab(0, 1)
            emit_u_chunk(2)
            for s in range(2, n_slabs):
                emit_x_slab(0, s)
            emit_r_memsets(1)

            # ---- fused epilogue clamp params (host-computed) [128, 9] ----
            bt = persist.tile([128, 9], F32, tag="bias", name="bias")
            nc.sync.dma_start(bt[:], b_d[:, :])

            # ---------------- main GEMM + epilogue loop -------------------
            def rhs(i, c, blk, kh, j, lo=0, hi=128):
                r0 = blk * RB
                return rq[(i, c)][lo:hi, j : j + 1, r0 + kh : r0 + kh + RB, :]

            def rhs_shift(i, blk, kh, j):
                # parts 64:128 of rq[(i,2)] hold R[r+1]: slice at kh-1
                r0 = blk * RB
                return rq[(i, 2)][
                    64:128, j : j + 1, r0 + kh - 1 : r0 + kh - 1 + RB, :
                ]

            def emit_epilogue(i, cot, ps, blk_of_half):
                """ps: dict j -> psum tile. One chain handles all listed
                partition-halves at once (cot2 pairs share the chain; bias
                cols are built per-partition-half). even = M0+M1+M2,
                odd = M1-M2-M3."""
                co0, cs = CHUNKS[cot]
                p0 = min(h[0] for h in blk_of_half)
                p1 = max(h[1] for h in blk_of_half)
                sl = slice(p0, p1)
                ot = opool.tile([128, RB * W], F32, tag="ot", name="ot")
                ov = ot.rearrange("p (r t two) -> p r t two", two=2, t=TW)
                te = epi.tile([128, RB * TW], F32, tag="te", name="te")
                to = epi.tile([128, RB * TW], F32, tag="to", name="to")
                c1 = epi.tile([128, RB * TW], F32, tag="c1", name="c1")
                c2 = epi.tile([128, RB * TW], F32, tag="c2", name="c2")
                nc.scalar.copy(c1[sl, :], ps[1][sl, :])
                nc.scalar.copy(c2[sl, :], ps[2][sl, :])
                nc.vector.tensor_tensor(
                    te[sl, :], c1[sl, :], ps[0][sl, :], OP.add
                )
                nc.gpsimd.tensor_tensor(
                    te[sl, :], te[sl, :], c2[sl, :], OP.add
                )
                nc.gpsimd.tensor_tensor(
                    to[sl, :], c1[sl, :], c2[sl, :], OP.subtract
                )
                nc.vector.tensor_tensor(
                    to[sl, :], to[sl, :], ps[3][sl, :], OP.subtract
                )
                for src, par in ((te, 0), (to, 1)):
                    # round(y*ss) via magic on ACT, then the fused
                    # clip+bias+clip as a 2-op per-partition clamp
                    nc.scalar.activation(
                        src[sl, :], src[sl, :], ACTF.Copy,
                        bias=MAGIC, scale=ss_h,
                    )
                    nc.vector.tensor_scalar(
                        src[sl, :], src[sl, :],
                        bt[sl, cot : cot + 1],
                        bt[sl, 3 + cot : 4 + cot],
                        OP.subtract, OP.min,
                    )
                    sv = src.rearrange("p (r t) -> p r t", t=TW)
                    nc.vector.tensor_scalar(
                        ov[sl, :, :, par : par + 1], sv[sl, :, :],
                        bt[sl, 6 + cot : 7 + cot], None, OP.max,
                    )
                for h0, h1, blk in blk_of_half:
                    r0 = blk * RB
                    nc.sync.dma_start(
                        y_d[i, co0 : co0 + cs, r0 * W : (r0 + RB) * W],
                        ot[h0:h1, :],
                    )

            for i in range(IMGS_PER_CORE):
                for cot, (co0, cs) in enumerate(CHUNKS):
                    if i == 0:
                        # trickle image-1 slab loads between image-0's GEMM
                        # sections (their DVE quant/R work would otherwise
                        # sit ahead of image-0 epilogue ops in the in-order
                        # engine queues and stall PSUM drain)
                        for s_ in range(cot, n_slabs, len(CHUNKS)):
                            emit_x_slab(1, s_)
                    if cs == 128:
                        for blk in range(NBLK):
                            ps = {
                                j: psum.tile([128, RB * TW], F32, tag="ps",
                                             name=f"ps{j}")
                                for j in range(J)
                            }
                            # c-major so later chunks' loads can trail
                            for c in (0, 1):
                                for j in range(J):
                                    for kh in range(K):
                                        nc.tensor.matmul(
                                            ps[j][:cs, :],
                                            uq[c][:, slot(kh, j) + co0 :
                                                  slot(kh, j) + co0 + cs],
                                            rhs(i, c, blk, kh, j),
                                            start=(c == 0 and kh == 0),
                                            stop=False,
                                        )
                            # ci-remainder: kh0+kh1 packed via shifted copy
                            for j in range(J):
                                nc.tensor.matmul(
                                    ps[j][:cs, :],
                                    uq[2][:, slot(0, j) + co0 :
                                          slot(0, j) + co0 + cs],
                                    rhs(i, 2, blk, 0, j, 0, 128),
                                    start=False,
                                    stop=False,
                                )
                            # kh=2 singles: (ja on rows 0:64, jb on 64:128)
                            # adjacent -> concurrent row tiles
                            for ja, jb in ((0, 1), (2, 3)):
                                nc.tensor.matmul(
                                    ps[ja][:cs, :],
                                    uq[2][0:64, slot(2, ja) + co0 :
                                          slot(2, ja) + co0 + cs],
                                    rhs(i, 2, blk, 2, ja, 0, 64),
                                    start=False,
                                    stop=True,
                                )
                                nc.tensor.matmul(
                                    ps[jb][:cs, :],
                                    uq[2][64:128, slot(1, jb) + co0 :
                                          slot(1, jb) + co0 + cs],
                                    rhs_shift(i, blk, 2, jb),
                                    start=False,
                                    stop=True,
                                )
                            emit_epilogue(i, cot, ps, [(0, 128, blk)])
                    else:
                        # cout remainder: col-pack row-block pairs into the
                        # two column halves of the array.
                        for q in range(NBLK // 2):
                            blkA, blkB = 2 * q, 2 * q + 1
                            ps = {
                                j: psum.tile([128, RB * TW], F32, tag="ps",
                                             name=f"ps{j}")
                                for j in range(J)
                            }
                            for c in (0, 1):
                                for j in range(J):
                                    for kh in range(K):
                                        w_ap = uq[c][:, slot(kh, j) + co0 :
                                                     slot(kh, j) + co0 + cs]
                                        st = c == 0 and kh == 0
                                        nc.tensor.matmul(
                                            ps[j][0:cs, :], w_ap,
                                            rhs(i, c, blkA, kh, j),
                                            start=st, stop=False,
                                            tile_position=(0, 0),
                                        )
                                        nc.tensor.matmul(
                                            ps[j][64 : 64 + cs, :], w_ap,
                                            rhs(i, c, blkB, kh, j),
                                            start=st, stop=False,
                                            tile_position=(0, 64),
                                        )
                            for j in range(J):
                                w_ap = uq[2][:, slot(0, j) + co0 :
                                             slot(0, j) + co0 + cs]
                                nc.tensor.matmul(
                                    ps[j][0:cs, :], w_ap,
                                    rhs(i, 2, blkA, 0, j, 0, 128),
                                    start=False, stop=False,
                                    tile_position=(0, 0),
                                )
                                nc.tensor.matmul(
                                    ps[j][64 : 64 + cs, :], w_ap,
                                    rhs(i, 2, blkB, 0, j, 0, 128),
                                    start=False, stop=False,
                                    tile_position=(0, 64),
                                )
                            # kh=2 singles: quadrant-packed (row, col)
                            for j in range(J):
                                nc.tensor.matmul(
                                    ps[j][0:cs, :],
                                    uq[2][0:64, slot(2, j) + co0 :
                                          slot(2, j) + co0 + cs],
                                    rhs(i, 2, blkA, 2, j, 0, 64),
                                    start=False, stop=True,
                                    tile_position=(0, 0),
                                )
                                nc.tensor.matmul(
                                    ps[j][64 : 64 + cs, :],
                                    uq[2][64:128, slot(1, j) + co0 :
                                          slot(1, j) + co0 + cs],
                                    rhs_shift(i, blkB, 2, j),
                                    start=False, stop=True,
                                    tile_position=(64, 64),
                                )
                            emit_epilogue(
                                i, cot, ps, [(0, 64, blkA), (64, 128, blkB)]
                            )

    nc.compile()
    return nc


_BUILD_CACHE = {}


def _get_nc(sx, sw, sb, ss):
    key = (sx, sw, sb, ss)
    if key not in _BUILD_CACHE:
        _BUILD_CACHE[key] = _build(sx, sw, sb, ss)
    return _BUILD_CACHE[key]


def _run(x, weight, bias, step_x, step_w, step_b, shift_scale, trace=False):
    _install_axon_ntff_hook()
    x = np.ascontiguousarray(np.asarray(x, dtype=np.float32))
    w = np.asarray(weight, dtype=np.float32)
    b = np.ascontiguousarray(np.asarray(bias, dtype=np.float32))
    sx = float(np.asarray(step_x))
    sw = float(np.asarray(step_w))
    sb = float(np.asarray(step_b))
    ss = float(np.asarray(shift_scale))

    nc = _get_nc(sx, sw, sb, ss)

    u_t = prep_u(w, sw)
    x_sh = x.reshape(N_CORES, IMGS_PER_CORE, CIN, HW)

    bp = bias_params(bias_int8(b, sb, ss, sx, sw))
    in_maps = [
        {"x": x_sh[core], "u": u_t, "b": bp} for core in range(N_CORES)
    ]
    res = run_bass_kernel_spmd(
        nc, in_maps, core_ids=list(range(N_CORES)), trace=trace
    )
    out = np.concatenate(
        [res.results[core]["y"].reshape(IMGS_PER_CORE, COUT, H, W)
         for core in range(N_CORES)],
        axis=0,
    )
    return out, res


def kernel(x, weight, bias, step_x, step_w, step_b, shift_scale):
    out, _ = _run(x, weight, bias, step_x, step_w, step_b, shift_scale)
    return out


def kernel_profiled(x, weight, bias, step_x, step_w, step_b, shift_scale):
    return _run(x, weight, bias, step_x, step_w, step_b, shift_scale, trace=True)


# revision 33
# speedup vs baseline: 1.0612x; 1.0612x over previous
"""Trainium2 Bass kernel for nn_Conv2d_lsq_int (LSQ int8-style quantized 3x3 conv).

Full-input contract: kernel(**inputs) takes the complete tensors
(x[16,320,64,64], weight[320,320,3,3], bias[320], scalar step sizes) and
returns the full [16,320,64,64] float32 output.

Distribution: data-parallel over the batch dim — 2 images per NeuronCore on
8 cores; weight/bias replicated. The host only shards the batch, quantizes +
Winograd-transforms the weight (pure weight preprocessing), computes the
320-element bias requant, and concatenates the per-core outputs.

Algorithm: 1D Winograd F(2,3) along the width dimension, fp16 operands.
 - x_int/w_int are integers in [-127,127]. Weight tap-rows g are transformed
   with 2*G = [[2,0,0],[1,1,1],[1,-1,1],[0,0,2]] -> U[kh,j] (ints, |U|<=381).
   Input rows are transformed with B^T -> R[j, tw] = [d0-d2, d1+d2, d2-d1,
   d1-d3] on even/odd column pairs (ints, |R|<=254). Both are exact in fp16
   (integers < 2048). Per output row-pair column tw: y[2tw+b] =
   (A^T M)[b] / 2 where M[j] = sum_{ci,kh} U[ci,kh,j] * R[ci, j, r+kh, tw].
 - MAC count per output: 4 j * 320 ci * 3 kh / 2 outputs = 1920 vs direct
   2880 -> 1.5x fewer tensor cycles. Products <= 97k and accumulated sums
   < 2^24, so fp32 PSUM accumulation is exact and the result matches the
   reference bit-for-bit (validated in numpy).
 - R layout [p, j, r, tw]: the GEMM moving operand (fixed j, 16 rows x 32
   tw) is one fully contiguous 1 KiB run per partition.
 - Contraction per (j, psum tile) = (ci 320 x kh 3) = 960 = 7.5*128: two full
   128-ci chunks x 3 kh, plus a 64-ci remainder packed as kh-pairs via a
   row-shifted partition-duplicated R copy (made by an SBUF DMA), plus K=64
   singles run pairwise concurrent via PE row tiling.
 - round() is fp32 add of 1.5*2**23 (round-to-nearest-even) fused into ACT
   activation ops; the epilogue clip+bias+clip collapses to a 2-op
   per-partition clamp (clip(clip(r)+b) == clamp(r+b, lo(b), hi(b))).
 - Engine budget: engines read at most one PSUM operand per instruction and
   gpsimd none, so ACT stages M1/M2 to SBUF, DVE does the PSUM-touching
   ops + R transform, gpsimd the SBUF-only combines.
"""

import contextlib
import ctypes
import sys
import types

import numpy as np

import concourse.bass as bass  # noqa: F401
import concourse.tile as tile
from concourse import bacc, mybir
from concourse.bass_utils import run_bass_kernel_spmd

F32 = mybir.dt.float32
F16 = mybir.dt.float16
OP = mybir.AluOpType
ACTF = mybir.ActivationFunctionType

MAGIC = 12582912.0  # 1.5 * 2**23 : fp32 round-to-nearest-even trick
QMAX = 127.0

B, CIN, COUT, H, W, K = 16, 320, 320, 64, 64, 3
N_CORES = 8
IMGS_PER_CORE = B // N_CORES
HW = H * W
J = 4            # winograd positions along width
TW = 32          # output column tiles (2 cols each)
PR = H + 2       # padded row count for R
RB = 16          # output rows per psum tile (N = RB*TW = 512)
NBLK = H // RB   # 4 row blocks per image
SLAB = 16        # x load/quant pipelined in 16-row slabs
CHUNKS = [(0, 128), (128, 128), (256, 64)]  # (start, size) along cin / cout
NSLOT = K * J    # 12 weight slots (kh, j)


def _install_axon_ntff_hook():
    """Slim antenv.axon_hooks so trace=True works (and never crashes) here."""
    if "antenv.axon_hooks" in sys.modules:
        return
    hook = None
    try:
        lib = ctypes.CDLL("/opt/axon/libaxon_pjrt.so")
        if hasattr(lib, "axon_start_nrt_profile"):
            lib.axon_start_nrt_profile.argtypes = [
                ctypes.POINTER(ctypes.c_int64),
                ctypes.c_size_t,
            ]
            lib.axon_start_nrt_profile.restype = ctypes.c_int64
            lib.axon_stop_nrt_profile.argtypes = [ctypes.c_char_p]
            lib.axon_stop_nrt_profile.restype = ctypes.c_int64

            @contextlib.contextmanager
            def hook(output_dir, device_ids):  # noqa: F811
                import jax

                jax.devices()
                if device_ids:
                    ids = (ctypes.c_int64 * len(device_ids))(*device_ids)
                    rc = lib.axon_start_nrt_profile(ids, len(device_ids))
                else:
                    rc = lib.axon_start_nrt_profile(None, 0)
                if rc != 0:
                    raise RuntimeError(f"axon_start_nrt_profile rc={rc}")
                try:
                    yield
                finally:
                    n = lib.axon_stop_nrt_profile(str(output_dir).encode())
                    print(f"profile: {n} ntff file(s) -> {output_dir}",
                          file=sys.stderr)
    except OSError:
        pass

    mod = types.ModuleType("antenv.axon_hooks")
    mod.get_axon_ntff_profile_hook = lambda: hook
    mod.set_axon_ntff_profile_hook = lambda h: None
    sys.modules["antenv.axon_hooks"] = mod

    # keep profiling artifacts local (zero-egress container)
    import concourse.bass_utils as bu

    bu.upload_artifacts = lambda tmpdir: "local://" + str(tmpdir)


def bias_int8(b, sb, ss, sx, sw):
    """Host fp32 replica of the reference's bias requant (DVE lacks divide).

    Every op is a single IEEE-754 fp32 operation in the reference's exact
    order, so this is bit-identical to the jax fp32 computation.
    """
    f32 = np.float32
    b = np.asarray(b, np.float32)
    b_deq = np.clip(np.round(b / f32(sb)), -QMAX, QMAX).astype(np.float32) * f32(sb)
    x_scale = f32(1.0) / f32(sx)
    w_scale = f32(1.0) / f32(sw)
    t = ((b_deq * f32(ss)) * x_scale) * w_scale
    return np.clip(np.round(t), -QMAX, QMAX).astype(np.float32)


def bias_params(b_i8):
    """Fused epilogue clamp params: clip(clip(r,+-127)+b, +-127) ==
    clamp(r+b, -127+max(0,b), 127+min(0,b)) since |b|<=127. Shipped as
    [128, 9]: cols p*3+c for param p in (MAGIC-b, hi, lo) and cout chunk
    c in (0:128, 128:256, 256:320 tiled on both partition halves)."""
    cols = np.zeros((128, 9), np.float32)
    vs = [b_i8[0:128], b_i8[128:256], np.tile(b_i8[256:320], 2)]
    for c, v in enumerate(vs):
        cols[:, 0 * 3 + c] = np.float32(MAGIC) - v
        cols[:, 1 * 3 + c] = QMAX + np.minimum(0.0, v)
        cols[:, 2 * 3 + c] = -QMAX + np.maximum(0.0, v)
    return cols


def prep_u(w, sw):
    """Host weight preprocessing: quantize + 1D Winograd transform (2G over
    kw), laid out [ci, (kh, j, cout)] in fp16 (all values are ints <= 381,
    exactly representable)."""
    f32 = np.float32
    w = np.asarray(w, np.float32)
    w_int = np.clip(np.round(w / f32(sw)), -QMAX, QMAX).astype(np.float32)
    g0, g1, g2 = w_int[..., 0], w_int[..., 1], w_int[..., 2]
    # [o, ci, kh, j]
    u = np.stack([2 * g0, g0 + g1 + g2, g0 - g1 + g2, 2 * g2], axis=-1)
    # -> [ci, kh, j, o] -> [CIN, 12*COUT]
    u = np.ascontiguousarray(u.transpose(1, 2, 3, 0)).reshape(CIN, NSLOT * COUT)
    return u.astype(np.float16)


def _build(sx: float, sw: float, sb: float, ss: float):
    """Build the per-core Bass program. Scalars are baked as immediates."""
    nc = bacc.Bacc("TRN2", target_bir_lowering=False, debug=False)

    x_d = nc.dram_tensor("x", [IMGS_PER_CORE, CIN, HW], F32, kind="ExternalInput")
    u_d = nc.dram_tensor("u", [CIN, NSLOT * COUT], F16, kind="ExternalInput")
    b_d = nc.dram_tensor("b", [128, 9], F32, kind="ExternalInput")
    y_d = nc.dram_tensor("y", [IMGS_PER_CORE, COUT, HW], F32, kind="ExternalOutput")

    r_x = float(np.float32(1.0) / np.float32(sx))  # x_scale
    # epilogue scale: shift_scale applied to y = (A^T M)/2
    ss_h = float(np.float32(ss)) / 2.0

    n_slabs = H // SLAB

    def slot(kh, j):
        return (kh * J + j) * COUT

    with tile.TileContext(nc) as tc:
        with (
            tc.tile_pool(name="persist", bufs=1) as persist,
            tc.tile_pool(name="xstage", bufs=3) as xstage,
            tc.tile_pool(name="epi", bufs=8) as epi,
            tc.tile_pool(name="otile", bufs=2) as opool,
            tc.tile_pool(name="psum", bufs=8, space="PSUM") as psum,
        ):
            # ---------------- weights: fp16 winograd U, DMA only ----------
            uq = {}
            half = (NSLOT // 2) * COUT
            for c in range(len(CHUNKS)):
                uq[c] = persist.tile(
                    [128, NSLOT * COUT], F16, tag=f"uq{c}", name=f"uq{c}"
                )

            def emit_u_chunk(c):
                ci0, pc = CHUNKS[c]
                if pc == 128:
                    for lo, hi in ((0, half), (half, NSLOT * COUT)):
                        nc.sync.dma_start(
                            uq[c][:, lo:hi], u_d[ci0 : ci0 + pc, lo:hi]
                        )
                else:
                    # parts 0:64 hold U[ci, kh, j]; parts 64:128 hold
                    # U[ci, kh+1, j] (slot-shifted) for kh-pair packing.
                    nc.sync.dma_start(
                        uq[c][:pc, :], u_d[ci0 : ci0 + pc, :]
                    )
                    nc.sync.dma_start(
                        uq[c][pc : 2 * pc, 0 : 2 * J * COUT],
                        u_d[ci0 : ci0 + pc, J * COUT : K * J * COUT],
                    )

            # ---------------- R: winograd-transformed input rows ----------
            # rq[(i,c)] viewed [p, j(4), r(66), tw(32)] fp16.
            # c=2: parts 0:64 = R rows as-is; parts 64:128 = R shifted one
            # row up (row r holds R[r+1]) for kh-pair / single packing.
            rq = {}
            for i in range(IMGS_PER_CORE):
                for c in range(len(CHUNKS)):
                    t = persist.tile(
                        [128, J * PR * TW], F16, tag=f"rq{i}_{c}",
                        name=f"rq{i}_{c}"
                    )
                    rq[(i, c)] = t.rearrange("p (j r t) -> p j r t", j=J, r=PR)

            def emit_r_memsets(i):
                for c in range(len(CHUNKS)):
                    r3 = rq[(i, c)]
                    if CHUNKS[c][1] == 128:
                        nc.gpsimd.memset(r3[:, :, 0:1, :], 0.0)
                        nc.gpsimd.memset(r3[:, :, PR - 1 :, :], 0.0)
                    else:
                        nc.gpsimd.memset(r3[0:64, :, 0:1, :], 0.0)
                        nc.gpsimd.memset(r3[0:64, :, PR - 1 :, :], 0.0)
                        nc.gpsimd.memset(r3[64:128, :, PR - 2 : PR - 1, :], 0.0)

            def emit_x_slab(i, s, only_c=None):
                """DMA a 16-row slab, quantize (round+clip in magic space),
                then write the width-transformed R rows directly. The
                remainder chunk's row-shifted duplicate (parts 64:128) is a
                partition-offset SBUF DMA copy of the computed R rows."""
                r0 = s * SLAB
                for c, (ci0, pc) in enumerate(CHUNKS):
                    if only_c is not None and c != only_c:
                        continue
                    st = xstage.tile([128, SLAB * W], F32, tag="xst",
                                     name="xst")
                    nc.sync.dma_start(
                        st[:pc, :],
                        x_d[i, ci0 : ci0 + pc, r0 * W : (r0 + SLAB) * W],
                    )
                    nc.scalar.activation(
                        st[:pc, :], st[:pc, :], ACTF.Copy, bias=MAGIC, scale=r_x
                    )
                    # clip in magic-offset space, single dual-op on DVE
                    nc.vector.tensor_scalar(
                        st[:pc, :], st[:pc, :], MAGIC + QMAX, MAGIC - QMAX,
                        OP.min, OP.max,
                    )
                    # even/odd column view: [p, r, tw(32), 2]
                    xv = st.rearrange("p (r c2 two) -> p r c2 two", two=2, c2=TW)
                    ev = lambda a, b: xv[0:pc, :, a:b, 0:1]
                    od = lambda a, b: xv[0:pc, :, a:b, 1:2]
                    d = rq[(i, c)]
                    rows = slice(1 + r0, 1 + r0 + SLAB)
                    # R on DVE (gpsimd's elementwise path is ~2.6x slower
                    # and fp16 slower still); magic offsets cancel in
                    # subtracts, j1 subtracts 2*MAGIC
                    ve = nc.vector
                    # j=0: R0[0] = -x[1]; R0[tw>=1] = x[2tw-1]-x[2tw+1]
                    ve.tensor_scalar(
                        d[0:pc, 0:1, rows, 0:1], od(0, 1),
                        -1.0, MAGIC, OP.mult, OP.add,
                    )
                    ve.tensor_tensor(
                        d[0:pc, 0:1, rows, 1:TW],
                        od(0, TW - 1), od(1, TW),
                        OP.subtract,
                    )
                    # j=1: x[2tw] + x[2tw+1]  (magic: (a-2M)+b)
                    ve.scalar_tensor_tensor(
                        d[0:pc, 1:2, rows, :],
                        ev(0, TW), 2.0 * MAGIC, od(0, TW),
                        OP.subtract, OP.add,
                    )
                    # j=2: x[2tw+1] - x[2tw]
                    ve.tensor_tensor(
                        d[0:pc, 2:3, rows, :],
                        od(0, TW), ev(0, TW),
                        OP.subtract,
                    )
                    # j=3: R3[tw<31] = x[2tw]-x[2tw+2]; R3[31] = x[62]
                    ve.tensor_tensor(
                        d[0:pc, 3:4, rows, 0 : TW - 1],
                        ev(0, TW - 1), ev(1, TW),
                        OP.subtract,
                    )
                    ve.tensor_scalar(
                        d[0:pc, 3:4, rows, TW - 1 : TW],
                        ev(TW - 1, TW), MAGIC, None, OP.subtract,
                    )
                    if pc < 128:
                        # row-shifted duplicate: R2b[r] = R[r+1]
                        nc.sync.dma_start(
                            d[64:128, :, r0 : r0 + SLAB, :],
                            d[0:64, :, 1 + r0 : 1 + r0 + SLAB, :],
                        )

            # emission order: first-consumption order — slab-0 chunk-0 DMA
            # goes out ahead of the 2.5 MB weight DMA so the quant pipeline
            # starts immediately. Image 1's slabs are emitted later,
            # interleaved between image-0 GEMM sections, so their DVE
            # quant/R work doesn't sit ahead of image-0 epilogue ops in the
            # in-order engine queues (that stalls PSUM drain).
            emit_x_slab(0, 0, only_c=0)
            emit_u_chunk(0)
            emit_x_slab(0, 0, only_c=1)
            emit_x_slab(0, 0, only_c=2)
            emit_u_chunk(1)
            emit_r_memsets(0)
            emit_x_slab(0, 1)
            emit_u_chunk(2)
            for s in range(2, n_slabs):
                emit_x_slab(0, s)
            emit_r_memsets(1)

            # ---- fused epilogue clamp params (host-computed) [128, 9] ----
            bt = persist.tile([128, 9], F32, tag="bias", name="bias")
            nc.sync.dma_start(bt[:], b_d[:, :])

            # ---------------- main GEMM + epilogue loop -------------------
            def rhs(i, c, blk, kh, j, lo=0, hi=128):
                r0 = blk * RB
                return rq[(i, c)][lo:hi, j : j + 1, r0 + kh : r0 + kh + RB, :]

            def rhs_shift(i, blk, kh, j):
                # parts 64:128 of rq[(i,2)] hold R[r+1]: slice at kh-1
                r0 = blk * RB
                return rq[(i, 2)][
                    64:128, j : j + 1, r0 + kh - 1 : r0 + kh - 1 + RB, :
                ]

            def emit_epilogue(i, cot, ps, blk_of_half):
                """ps: dict j -> psum tile. One chain handles all listed
                partition-halves at once (cot2 pairs share the chain; bias
                cols are built per-partition-half). even = M0+M1+M2,
                odd = M1-M2-M3."""
                co0, cs = CHUNKS[cot]
                p0 = min(h[0] for h in blk_of_half)
                p1 = max(h[1] for h in blk_of_half)
                sl = slice(p0, p1)
                ot = opool.tile([128, RB * W], F32, tag="ot", name="ot")
                ov = ot.rearrange("p (r t two) -> p r t two", two=2, t=TW)
                te = epi.tile([128, RB * TW], F32, tag="te", name="te")
                to = epi.tile([128, RB * TW], F32, tag="to", name="to")
                c1 = epi.tile([128, RB * TW], F32, tag="c1", name="c1")
                c2 = epi.tile([128, RB * TW], F32, tag="c2", name="c2")
                nc.scalar.copy(c1[sl, :], ps[1][sl, :])
                nc.scalar.copy(c2[sl, :], ps[2][sl, :])
                nc.vector.tensor_tensor(
                    te[sl, :], c1[sl, :], ps[0][sl, :], OP.add
                )
                nc.gpsimd.tensor_tensor(
                    te[sl, :], te[sl, :], c2[sl, :], OP.add
                )
                nc.gpsimd.tensor_tensor(
                    to[sl, :], c1[sl, :], c2[sl, :], OP.subtract
                )
                nc.vector.tensor_tensor(
                    to[sl, :], to[sl, :], ps[3][sl, :], OP.subtract
                )
                for src, par in ((te, 0), (to, 1)):
                    # round(y*ss) via magic on ACT, then the fused
                    # clip+bias+clip as a 2-op per-partition clamp
                    nc.scalar.activation(
                        src[sl, :], src[sl, :], ACTF.Copy,
                        bias=MAGIC, scale=ss_h,
                    )
                    nc.vector.tensor_scalar(
                        src[sl, :], src[sl, :],
                        bt[sl, cot : cot + 1],
                        bt[sl, 3 + cot : 4 + cot],
                        OP.subtract, OP.min,
                    )
                    sv = src.rearrange("p (r t) -> p r t", t=TW)
                    nc.vector.tensor_scalar(
                        ov[sl, :, :, par : par + 1], sv[sl, :, :],
                        bt[sl, 6 + cot : 7 + cot], None, OP.max,
                    )
                for h0, h1, blk in blk_of_half:
                    r0 = blk * RB
                    nc.sync.dma_start(
                        y_d[i, co0 : co0 + cs, r0 * W : (r0 + RB) * W],
                        ot[h0:h1, :],
                    )

            for i in range(IMGS_PER_CORE):
                for cot, (co0, cs) in enumerate(CHUNKS):
                    if i == 0:
                        # trickle image-1 slab loads between image-0's GEMM
                        # sections (their DVE quant/R work would otherwise
                        # sit ahead of image-0 epilogue ops in the in-order
                        # engine queues and stall PSUM drain)
                        for s_ in range(cot, n_slabs, len(CHUNKS)):
                            emit_x_slab(1, s_)
                    if cs == 128:
                        for blk in range(NBLK):
                            ps = {
                                j: psum.tile([128, RB * TW], F32, tag="ps",
                                             name=f"ps{j}")
                                for j in range(J)
                            }
                            # c-major so later chunks' loads can trail
                            for c in (0, 1):
                                for j in range(J):
                                    for kh in range(K):
                                        nc.tensor.matmul(
                                            ps[j][:cs, :],
                                            uq[c][:, slot(kh, j) + co0 :
                                                  slot(kh, j) + co0 + cs],
                                            rhs(i, c, blk, kh, j),
                                            start=(c == 0 and kh == 0),
                                            stop=False,
                                        )
                            # ci-remainder: kh0+kh1 packed via shifted copy
                            for j in range(J):
                                nc.tensor.matmul(
                                    ps[j][:cs, :],
                                    uq[2][:, slot(0, j) + co0 :
                                          slot(0, j) + co0 + cs],
                                    rhs(i, 2, blk, 0, j, 0, 128),
                                    start=False,
                                    stop=False,
                                )
                            # kh=2 singles: (ja on rows 0:64, jb on 64:128)
                            # adjacent -> concurrent row tiles
                            for ja, jb in ((0, 1), (2, 3)):
                                nc.tensor.matmul(
                                    ps[ja][:cs, :],
                                    uq[2][0:64, slot(2, ja) + co0 :
                                          slot(2, ja) + co0 + cs],
                                    rhs(i, 2, blk, 2, ja, 0, 64),
                                    start=False,
                                    stop=True,
                                )
                                nc.tensor.matmul(
                                    ps[jb][:cs, :],
                                    uq[2][64:128, slot(1, jb) + co0 :
                                          slot(1, jb) + co0 + cs],
                                    rhs_shift(i, blk, 2, jb),
                                    start=False,
                                    stop=True,
                                )
                            emit_epilogue(i, cot, ps, [(0, 128, blk)])
                    else:
                        # cout remainder: col-pack row-block pairs into the
                        # two column halves of the array.
                        for q in range(NBLK // 2):
                            blkA, blkB = 2 * q, 2 * q + 1
                            ps = {
                                j: psum.tile([128, RB * TW], F32, tag="ps",
                                             name=f"ps{j}")
                                for j in range(J)
                            }
                            for c in (0, 1):
                                for j in range(J):
                                    for kh in range(K):
                                        w_ap = uq[c][:, slot(kh, j) + co0 :
                                                     slot(kh, j) + co0 + cs]
                                        st = c == 0 and kh == 0
                                        nc.tensor.matmul(
                                            ps[j][0:cs, :], w_ap,
                                            rhs(i, c, blkA, kh, j),
                                            start=st, stop=False,
                                            tile_position=(0, 0),
                                        )
                                        nc.tensor.matmul(
                                            ps[j][64 : 64 + cs, :], w_ap,
                                            rhs(i, c, blkB, kh, j),
                                            start=st, stop=False,
                                            tile_position=(0, 64),
                                        )
                            for j in range(J):
                                w_ap = uq[2][:, slot(0, j) + co0 :
                                             slot(0, j) + co0 + cs]
                                nc.tensor.matmul(
                                    ps[j][0:cs, :], w_ap,
                                    rhs(i, 2, blkA, 0, j, 0, 128),
                                    start=False, stop=False,
                                    tile_position=(0, 0),
                                )
                                nc.tensor.matmul(
                                    ps[j][64 : 64 + cs, :], w_ap,
                                    rhs(i, 2, blkB, 0, j, 0, 128),
                                    start=False, stop=False,
                                    tile_position=(0, 64),
                                )
                            # kh=2 singles: quadrant-packed (row, col)
                            for j in range(J):
                                nc.tensor.matmul(
                                    ps[j][0:cs, :],
                                    uq[2][0:64, slot(2, j) + co0 :
                                          slot(2, j) + co0 + cs],
                                    rhs(i, 2, blkA, 2, j, 0, 64),
                                    start=False, stop=True,
                                    tile_position=(0, 0),
                                )
                                nc.tensor.matmul(
                                    ps[j][64 : 64 + cs, :],
                                    uq[2][64:128, slot(1, j) + co0 :
                                          slot(1, j) + co0 + cs],
                                    rhs_shift(i, blkB, 2, j),
                                    start=False, stop=True,
                                    tile_position=(64, 64),
                                )
                            emit_epilogue(
                                i, cot, ps, [(0, 64, blkA), (64, 128, blkB)]
                            )

    nc.compile()
    return nc


_BUILD_CACHE = {}


def _get_nc(sx, sw, sb, ss):
    key = (sx, sw, sb, ss)
    if key not in _BUILD_CACHE:
        _BUILD_CACHE[key] = _build(sx, sw, sb, ss)
    return _BUILD_CACHE[key]


def _run(x, weight, bias, step_x, step_w, step_b, shift_scale, trace=False):
    _install_axon_ntff_hook()
    x = np.ascontiguousarray(np.asarray(x, dtype=np.float32))
    w = np.asarray(weight, dtype=np.float32)
    b = np.ascontiguousarray(np.asarray(bias, dtype=np.float32))
    sx = float(np.asarray(step_x))
    sw = float(np.asarray(step_w))
    sb = float(np.asarray(step_b))
    ss = float(np.asarray(shift_scale))

    nc = _get_nc(sx, sw, sb, ss)

    u_t = prep_u(w, sw)
    x_sh = x.reshape(N_CORES, IMGS_PER_CORE, CIN, HW)

    bp = bias_params(bias_int8(b, sb, ss, sx, sw))
    in_maps = [
        {"x": x_sh[core], "u": u_t, "b": bp} for core in range(N_CORES)
    ]
    res = run_bass_kernel_spmd(
        nc, in_maps, core_ids=list(range(N_CORES)), trace=trace
    )
    out = np.concatenate(
        [res.results[core]["y"].reshape(IMGS_PER_CORE, COUT, H, W)
         for core in range(N_CORES)],
        axis=0,
    )
    return out, res


def kernel(x, weight, bias, step_x, step_w, step_b, shift_scale):
    out, _ = _run(x, weight, bias, step_x, step_w, step_b, shift_scale)
    return out


def kernel_profiled(x, weight, bias, step_x, step_w, step_b, shift_scale):
    return _run(x, weight, bias, step_x, step_w, step_b, shift_scale, trace=True)
